# revision 4
# baseline (speedup 1.0000x reference)
"""GAT (3-layer DGL-style GATConv) on 8 Trainium2 NeuronCores.

Strategy (graph/data parallel, dst-sharded):
  * dst nodes are sharded across the 8 cores (12500 each), grouped into
    128-dst blocks; per block, incoming edges are bucketed by src z-table
    chunk (int16 gather index range) into fixed-size slot groups.
  * Per layer, a "node" launch computes z = h @ W (plus the per-node
    attention terms el/er) sharded by node slice; the host concatenates the
    z slices (bf16 rows padded to a 256B-multiple, with el stored inside
    the row) and feeds the full z table to the "edge" launch.
  * The edge launch dma_gathers z rows by src edge-by-edge, computes
    ex = exp(leakyrelu(el_src + er_dst)) on-chip, and aggregates
    out[d] = sum_e ex_e * z[src_e] / sum_e ex_e with a mask matmul on the
    tensor engine (128-edge chunks against a 128-dst one-hot mask built
    with is_equal against an iota), accumulating numerator and denominator
    in PSUM in one pass.  Segment softmax needs no max subtraction (shift
    invariance; logits are O(8) here).
  * Head mean / ReLU / bias / final class softmax run fused in the edge
    launch epilogue.

kernel(**inputs) takes the FULL unsharded inputs and returns the FULL
[N, n_classes] float32 output.
"""

import math
import os
from dataclasses import dataclass, field

import numpy as np
import ml_dtypes

BF16 = ml_dtypes.bfloat16
P = 128
NCHUNK = 4  # z-table split so gather indices fit int16


# --------------------------------------------------------------------------
# host-side plan: dst->block packing, slot layout, gather index arrays
# --------------------------------------------------------------------------

@dataclass
class Plan:
    n_cores: int
    N: int
    ND: int          # dst nodes per core
    NB: int          # 128-dst blocks per core
    NT: int          # node tiles per core (= NB)
    CH: int          # z-chunk rows
    chunk_rows: list  # rows per z chunk
    G: int           # gather count per (block, chunk) group (mult of 128)
    SCc: int         # subchunks per chunk group (= G // 128)
    SC: int          # subchunks per block (= 4 * SCc)
    idx: list = field(default_factory=list)        # per core [P, NB*4*(G//16)] i16
    dl: list = field(default_factory=list)         # per core [P, NB*SC] bf16
    slot_dst: list = field(default_factory=list)   # per core [NB, SC, P] i32 (-1 pad)
    row2node: list = field(default_factory=list)   # per core [NB*P] i32 (-1 pad)


def build_plan(src, dst, N, n_cores):
    src = np.asarray(src).astype(np.int64)
    dst = np.asarray(dst).astype(np.int64)
    ND = N // n_cores
    assert ND * n_cores == N
    NB = (ND + P - 1) // P
    CH = (N + NCHUNK - 1) // NCHUNK
    chunk_rows = [min(CH, N - c * CH) for c in range(NCHUNK)]

    cores = []
    gmax = 0
    for k in range(n_cores):
        m = (dst >= k * ND) & (dst < (k + 1) * ND)
        dk = dst[m] - k * ND
        sk = src[m]
        deg = np.bincount(dk, minlength=ND)
        order = np.argsort(-deg, kind="stable")
        blk = np.empty(ND, np.int32)
        pos = np.empty(ND, np.int32)
        # snake-deal dsts (desc degree) into NB blocks to balance edge counts
        for i in range(0, ND, NB):
            ch = order[i : i + NB]
            r = i // NB
            if r % 2 == 0:
                b_ids = np.arange(len(ch))
            else:
                b_ids = NB - 1 - np.arange(len(ch))
            blk[ch] = b_ids
            pos[ch] = r
        chunk_id = (sk // CH).astype(np.int64)
        cores.append((dk, sk, blk, pos, chunk_id))
        cnt = np.bincount(blk[dk] * NCHUNK + chunk_id, minlength=NB * NCHUNK)
        gmax = max(gmax, int(cnt.max()))

    G = ((gmax + P - 1) // P) * P
    SCc = G // P
    SC = NCHUNK * SCc
    plan = Plan(n_cores, N, ND, NB, NB, CH, chunk_rows, G, SCc, SC)

    GC = G // 16
    for k in range(n_cores):
        dk, sk, blk, pos, chunk_id = cores[k]
        idx_arr = np.zeros((P, NB * NCHUNK * GC), np.int16)
        dl_arr = np.zeros((P, NB * SC), BF16)
        slot_dst = np.full((NB, SC, P), -1, np.int32)
        row2node = np.full(NB * P, -1, np.int32)

        # row2node: block b row p -> global dst node
        node_of = np.full((NB, P), -1, np.int64)
        node_of[blk, pos] = np.arange(ND)
        valid = node_of >= 0
        row2node[valid.ravel()] = (node_of[valid] + k * ND).astype(np.int32)

        # group edges by (block, chunk)
        key = blk[dk].astype(np.int64) * NCHUNK + chunk_id
        sort = np.argsort(key, kind="stable")
        ks = key[sort]
        dks = dk[sort]
        sks = sk[sort]
        starts = np.searchsorted(ks, np.arange(NB * NCHUNK))
        ends = np.searchsorted(ks, np.arange(NB * NCHUNK) + 1)
        for b in range(NB):
            for c in range(NCHUNK):
                g0, g1 = starts[b * NCHUNK + c], ends[b * NCHUNK + c]
                n = g1 - g0
                loc_idx = (sks[g0:g1] - c * CH).astype(np.int16)
                loc_dst = pos[dks[g0:g1]]
                # gather idxs: slot j -> [j%16, j//16] of a [16, GC] grid
                grid = np.zeros((16, GC), np.int16)
                flat = np.zeros(G, np.int16)
                flat[:n] = loc_idx
                grid[:, :] = flat.reshape(GC, 16).T
                idx_arr[:, (b * NCHUNK + c) * GC : (b * NCHUNK + c + 1) * GC] = (
                    np.tile(grid, (8, 1))
                )
                # slot s of group -> subchunk kk = c*SCc + s//128, partition s%128
                s = np.arange(n)
                kk = c * SCc + s // P
                pp = s % P
                dl_arr[pp, b * SC + kk] = loc_dst.astype(BF16)
                slot_dst[b, kk, pp] = dks[g0:g1] + k * ND
        plan.idx.append(idx_arr)
        plan.dl.append(dl_arr)
        plan.slot_dst.append(slot_dst)
        plan.row2node.append(row2node)
    return plan


# --------------------------------------------------------------------------
# bass program builders
# --------------------------------------------------------------------------

def _bass_mods():
    import concourse.bass as bass
    import concourse.bacc as bacc
    import concourse.tile as tile
    import concourse.mybir as mybir
    return bass, bacc, tile, mybir


def build_node_program(Din, HF, R, NT):
    """z = hT.T @ W  (+ el/er attention terms).  Outputs z rows (bf16,
    width R, el packed as f32 at byte offset 2*HF) and er (f32)."""
    bass, bacc, tile, mybir = _bass_mods()
    f32, bf16 = mybir.dt.float32, mybir.dt.bfloat16
    H = 8
    F = HF // H
    KC = (Din + P - 1) // P

    nc = bacc.Bacc("TRN2", target_bir_lowering=False, debug=False)
    hT = nc.dram_tensor("hT", [Din, NT * P], bf16, kind="ExternalInput").ap()
    W = nc.dram_tensor("W", [Din, HF], bf16, kind="ExternalInput").ap()
    al = nc.dram_tensor("al", [P, HF], f32, kind="ExternalInput").ap()
    ar = nc.dram_tensor("ar", [P, HF], f32, kind="ExternalInput").ap()
    z_out = nc.dram_tensor("z_out", [NT * P, R], bf16, kind="ExternalOutput").ap()
    er_out = nc.dram_tensor("er_out", [NT * P, 8], f32, kind="ExternalOutput").ap()

    with tile.TileContext(nc) as tc:
        from contextlib import ExitStack
        with ExitStack() as ctx:
            cpool = ctx.enter_context(tc.tile_pool(name="const", bufs=1))
            lpool = ctx.enter_context(tc.tile_pool(name="lhs", bufs=4))
            zpool = ctx.enter_context(tc.tile_pool(name="z", bufs=3))
            spool = ctx.enter_context(tc.tile_pool(name="small", bufs=4))
            ppool = ctx.enter_context(tc.tile_pool(name="psum", bufs=2, space="PSUM"))

            W_t = []
            for kc in range(KC):
                K = min(P, Din - kc * P)
                wt = cpool.tile([K, HF], bf16, tag=f"w{kc}")
                nc.sync.dma_start(wt[:], W[kc * P : kc * P + K, :])
                W_t.append(wt)
            al_t = cpool.tile([P, HF], f32, tag="al")
            nc.sync.dma_start(al_t[:], al[:])
            ar_t = cpool.tile([P, HF], f32, tag="ar")
            nc.sync.dma_start(ar_t[:], ar[:])

            for t in range(NT):
                ps = ppool.tile([P, HF], f32, tag="psz")
                for kc in range(KC):
                    K = min(P, Din - kc * P)
                    lh = lpool.tile([K, P], bf16, tag="lh")
                    nc.sync.dma_start(
                        lh[:], hT[kc * P : kc * P + K, t * P : (t + 1) * P]
                    )
                    nc.tensor.matmul(
                        ps[:], lhsT=lh[:], rhs=W_t[kc][:],
                        start=(kc == 0), stop=(kc == KC - 1),
                    )
                zrow = zpool.tile([P, R], bf16, tag="zrow")
                nc.scalar.activation(
                    zrow[:, 0:HF], ps[:], mybir.ActivationFunctionType.Copy
                )
                if R > HF + 16:
                    nc.vector.memset(zrow[:, HF + 16 : R], 0)
                tmp = zpool.tile([P, HF], f32, tag="tmp")
                nc.vector.tensor_tensor(
                    out=tmp[:], in0=ps[:], in1=al_t[:], op=mybir.AluOpType.mult
                )
                el = spool.tile([P, 8], f32, tag="el")
                nc.vector.reduce_sum(
                    el[:], tmp[:].rearrange("p (h f) -> p h f", f=F),
                    axis=mybir.AxisListType.X,
                )
                nc.vector.tensor_tensor(
                    out=tmp[:], in0=ps[:], in1=ar_t[:], op=mybir.AluOpType.mult
                )
                er = spool.tile([P, 8], f32, tag="er")
                nc.vector.reduce_sum(
                    er[:], tmp[:].rearrange("p (h f) -> p h f", f=F),
                    axis=mybir.AxisListType.X,
                )
                # pack el (f32) into the z row at byte offset 2*HF
                zf = zrow[:].bitcast(f32)
                nc.vector.tensor_copy(out=zf[:, HF // 2 : HF // 2 + 8], in_=el[:])
                nc.sync.dma_start(z_out[t * P : (t + 1) * P, :], zrow[:])
                nc.sync.dma_start(er_out[t * P : (t + 1) * P, :], er[:])
    nc.compile()
    return nc


def build_edge_program(HF, R, NB, SC, SCc, G, chunk_rows, final, n_classes=41):
    """Gather z rows by src, segment-softmax aggregate per dst block."""
    bass, bacc, tile, mybir = _bass_mods()
    f32, bf16, i16 = mybir.dt.float32, mybir.dt.bfloat16, mybir.dt.int16
    H = 8
    F = HF // H
    GC = G // 16

    nc = bacc.Bacc("TRN2", target_bir_lowering=False, debug=False)
    zc = [
        nc.dram_tensor(f"z{c}", [chunk_rows[c], R], bf16, kind="ExternalInput").ap()
        for c in range(NCHUNK)
    ]
    idx = nc.dram_tensor("idx", [P, NB * NCHUNK * GC], i16, kind="ExternalInput").ap()
    dl = nc.dram_tensor("dl", [P, NB * SC], bf16, kind="ExternalInput").ap()
    ere = nc.dram_tensor("ere", [P, NB * SC * 8], f32, kind="ExternalInput").ap()
    iota = nc.dram_tensor("iota", [P, P], bf16, kind="ExternalInput").ap()
    brep = nc.dram_tensor("brep", [P, HF], f32, kind="ExternalInput").ap()
    OW = n_classes if final else F
    out = nc.dram_tensor("out", [NB * P, OW], f32, kind="ExternalOutput").ap()

    with tile.TileContext(nc) as tc:
        from contextlib import ExitStack
        with ExitStack() as ctx:
            cpool = ctx.enter_context(tc.tile_pool(name="const", bufs=1))
            gpool = ctx.enter_context(tc.tile_pool(name="gath", bufs=2))
            mpool = ctx.enter_context(tc.tile_pool(name="mask", bufs=2))
            spool = ctx.enter_context(tc.tile_pool(name="small", bufs=3))
            opool = ctx.enter_context(tc.tile_pool(name="outs", bufs=3))
            ppool = ctx.enter_context(tc.tile_pool(name="psum", bufs=2, space="PSUM"))

            iota_t = cpool.tile([P, P], bf16, tag="iota")
            nc.sync.dma_start(iota_t[:], iota[:])
            b_t = cpool.tile([P, HF], f32, tag="brep")
            nc.sync.dma_start(b_t[:], brep[:])

            for b in range(NB):
                idx_t = spool.tile([P, NCHUNK * GC], i16, tag="idx")
                nc.sync.dma_start(
                    idx_t[:], idx[:, b * NCHUNK * GC : (b + 1) * NCHUNK * GC]
                )
                dl_t = spool.tile([P, SC], bf16, tag="dl")
                nc.sync.dma_start(dl_t[:], dl[:, b * SC : (b + 1) * SC])
                er_t = spool.tile([P, SC * 8], f32, tag="ere")
                nc.sync.dma_start(er_t[:], ere[:, b * SC * 8 : (b + 1) * SC * 8])

                Zg = gpool.tile([P, SC, R], bf16, tag="Zg")
                for c in range(NCHUNK):
                    nc.gpsimd.dma_gather(
                        Zg[:, c * SCc : (c + 1) * SCc, :],
                        zc[c][:],
                        idx_t[:, c * GC : (c + 1) * GC],
                        num_idxs=G,
                        num_idxs_reg=G,
                        elem_size=R,
                        elem_step=R,
                    )
                # e = el_src + er_dst ; el is f32 packed in the row
                elv = (
                    Zg[:]
                    .rearrange("p k r -> p (k r)")
                    .bitcast(f32)
                    .rearrange("p (k m) -> p k m", m=R // 2)[
                        :, :, HF // 2 : HF // 2 + 8
                    ]
                )
                e_t = spool.tile([P, SC * 8], f32, tag="e")
                nc.vector.tensor_tensor(
                    out=e_t[:].rearrange("p (k h) -> p k h", h=8),
                    in0=elv,
                    in1=er_t[:].rearrange("p (k h) -> p k h", h=8),
                    op=mybir.AluOpType.add,
                )
                # leaky relu: max(e, 0.2*e)
                elr = spool.tile([P, SC * 8], f32, tag="elr")
                nc.vector.scalar_tensor_tensor(
                    out=elr[:], in0=e_t[:], scalar=0.2, in1=e_t[:],
                    op0=mybir.AluOpType.mult, op1=mybir.AluOpType.max,
                )
                exb = spool.tile([P, SC * 8], bf16, tag="exb")
                nc.scalar.activation(
                    exb[:], elr[:], mybir.ActivationFunctionType.Exp
                )
                # one-hot dst masks
                masks = mpool.tile([P, SC, P], bf16, tag="masks")
                nc.vector.tensor_tensor(
                    out=masks[:],
                    in0=dl_t[:].unsqueeze(2).to_broadcast([P, SC, P]),
                    in1=iota_t[:].unsqueeze(1).to_broadcast([P, SC, P]),
                    op=mybir.AluOpType.is_equal,
                )
                # scale gathered z rows by ex (in place)
                nc.vector.tensor_tensor(
                    out=Zg[:, :, 0:HF].rearrange("p k (h f) -> p k h f", f=F),
                    in0=Zg[:, :, 0:HF].rearrange("p k (h f) -> p k h f", f=F),
                    in1=exb[:]
                    .rearrange("p (k h) -> p k h", h=8)
                    .unsqueeze(3)
                    .to_broadcast([P, SC, 8, F]),
                    op=mybir.AluOpType.mult,
                )
                ps_n = ppool.tile([P, HF], f32, tag="psn")
                ps_s = ppool.tile([P, 8], f32, tag="pss")
                for k in range(SC):
                    nc.tensor.matmul(
                        ps_n[:], lhsT=masks[:, k, :], rhs=Zg[:, k, 0:HF],
                        start=(k == 0), stop=(k == SC - 1),
                    )
                    nc.tensor.matmul(
                        ps_s[:], lhsT=masks[:, k, :],
                        rhs=exb[:, k * 8 : (k + 1) * 8],
                        start=(k == 0), stop=(k == SC - 1),
                    )
                srm = spool.tile([P, 8], f32, tag="srm")
                nc.vector.tensor_scalar_max(out=srm[:], in0=ps_s[:], scalar1=1e-12)
                srec = spool.tile([P, 8], f32, tag="srec")
                nc.vector.reciprocal(out=srec[:], in_=srm[:])
                outg = opool.tile([P, HF], f32, tag="outg")
                nc.vector.tensor_tensor(
                    out=outg[:].rearrange("p (h f) -> p h f", f=F),
                    in0=ps_n[:].rearrange("p (h f) -> p h f", f=F),
                    in1=srec[:].unsqueeze(2).to_broadcast([P, 8, F]),
                    op=mybir.AluOpType.mult,
                )
                nc.vector.tensor_tensor(
                    out=outg[:], in0=outg[:], in1=b_t[:], op=mybir.AluOpType.add
                )
                if not final:
                    r = opool.tile([P, HF], f32, tag="r")
                    nc.scalar.activation(
                        r[:], outg[:], mybir.ActivationFunctionType.Relu,
                        scale=0.125,
                    )
                    ht = opool.tile([P, F], f32, tag="ht")
                    nc.vector.reduce_sum(
                        ht[:],
                        r[:].rearrange("p (h f) -> p f h", f=F),
                        axis=mybir.AxisListType.X,
                    )
                    nc.sync.dma_start(out[b * P : (b + 1) * P, :], ht[:])
                else:
                    q = opool.tile([P, n_classes], f32, tag="q")
                    nc.vector.reduce_sum(
                        q[:],
                        outg[:].rearrange("p (h f) -> p f h", f=F),
                        axis=mybir.AxisListType.X,
                    )
                    qm = spool.tile([P, 1], f32, tag="qm")
                    nc.vector.reduce_max(qm[:], q[:], axis=mybir.AxisListType.X)
                    negm = spool.tile([P, 1], f32, tag="negm")
                    nc.vector.tensor_scalar_mul(out=negm[:], in0=qm[:], scalar1=-0.125)
                    qe = opool.tile([P, n_classes], f32, tag="qe")
                    nc.scalar.activation(
                        qe[:], q[:], mybir.ActivationFunctionType.Exp,
                        bias=negm[:], scale=0.125,
                    )
                    qs = spool.tile([P, 1], f32, tag="qs")
                    nc.vector.reduce_sum(qs[:], qe[:], axis=mybir.AxisListType.X)
                    qsr = spool.tile([P, 1], f32, tag="qsr")
                    nc.vector.reciprocal(out=qsr[:], in_=qs[:])
                    outf = opool.tile([P, n_classes], f32, tag="outf")
                    nc.vector.tensor_single_scalar(
                        out=outf[:], in_=qe[:], scalar=qsr[:],
                        op=mybir.AluOpType.mult,
                    )
                    nc.sync.dma_start(out[b * P : (b + 1) * P, :], outf[:])
    nc.compile()
    return nc


# --------------------------------------------------------------------------
# orchestration
# --------------------------------------------------------------------------

_PROG_CACHE = {}
LAST_RUN_NS = []  # per-launch max-core exec ns when GAT_TRACE=1


def _get_prog(key, builder):
    if key not in _PROG_CACHE:
        _PROG_CACHE[key] = builder()
    return _PROG_CACHE[key]


def _run(nc, in_maps, n_cores):
    if os.environ.get("GAT_SIM", "0") == "1":
        return _run_sim(nc, in_maps)
    from concourse.bass_utils import run_bass_kernel_spmd

    trace = os.environ.get("GAT_TRACE", "0") == "1"
    core_ids = list(range(n_cores))
    res = run_bass_kernel_spmd(
        nc, in_maps, core_ids,
        trace=trace, trace_cores=core_ids if trace else None,
    )
    if trace:
        LAST_RUN_NS.append(res.exec_time_ns)
    return res.results


def _run_sim(nc, in_maps):
    """CoreSim (functional simulator) execution, one core at a time."""
    from concourse.bass_interp import CoreSim

    results = []
    for im in in_maps:
        sim = CoreSim(nc, trace=False, require_finite=False, require_nnan=False)
        for name, arr in im.items():
            sim.tensor(name)[:] = arr
        sim.simulate(check_with_hw=False)
        out = {}
        for alloc in nc.m.functions[0].allocations:
            import concourse.mybir as mybir
            if (
                isinstance(alloc, mybir.MemoryLocationSet)
                and alloc.kind == "ExternalOutput"
            ):
                name = alloc.memorylocations[0].name
                out[name] = np.array(sim.tensor(name))
        results.append(out)
    return results


def _row_pack(el_hf_bytes):
    pass


def gat_forward(x, src, dst, params, N=None, n_cores=8, n_classes=41):
    """params: list of 3 dicts with W [Din, H*F], al/ar [H, F], b [H, F]."""
    N = N if N is not None else x.shape[0]
    H = 8
    plan = build_plan(src, dst, N, n_cores)
    NB, NT, SC, SCc, G, CH = plan.NB, plan.NT, plan.SC, plan.SCc, plan.G, plan.CH
    GC = G // 16
    iota = np.tile(np.arange(P, dtype=np.float32).astype(BF16)[None, :], (P, 1))

    layer_dims = []
    for li, prm in enumerate(params):
        Din = prm["W"].shape[0]
        F = prm["al"].shape[1]
        HF = H * F
        # bf16 row: z (HF) + el f32 (16 bytes) padded to 256B multiple
        R = ((HF * 2 + 32 + 255) // 256) * 256 // 2
        layer_dims.append((Din, F, HF, R))

    h = np.asarray(x, np.float32)
    out_final = None
    for li, prm in enumerate(params):
        Din, F, HF, R = layer_dims[li]
        final = li == len(params) - 1

        node_nc = _get_prog(
            ("node", Din, HF, R, NT), lambda: build_node_program(Din, HF, R, NT)
        )
        # node launch inputs
        W_bf = prm["W"].astype(BF16)
        al_rep = np.tile(prm["al"].reshape(1, HF).astype(np.float32), (P, 1))
        ar_rep = np.tile(prm["ar"].reshape(1, HF).astype(np.float32), (P, 1))
        in_maps = []
        for k in range(n_cores):
            hk = h[k * plan.ND : (k + 1) * plan.ND]
            hT = np.zeros((Din, NT * P), BF16)
            hT[:, : plan.ND] = hk.T.astype(BF16)
            in_maps.append({"hT": hT, "W": W_bf, "al": al_rep, "ar": ar_rep})
        res = _run(node_nc, in_maps, n_cores)

        z_full = np.concatenate(
            [res[k]["z_out"][: plan.ND] for k in range(n_cores)], axis=0
        )
        er_full = np.concatenate(
            [res[k]["er_out"][: plan.ND] for k in range(n_cores)], axis=0
        )

        edge_nc = _get_prog(
            ("edge", HF, R, NB, SC, G, final, tuple(plan.chunk_rows)),
            lambda: build_edge_program(
                HF, R, NB, SC, SCc, G, plan.chunk_rows, final, n_classes
            ),
        )
        b_rep = np.tile(prm["b"].reshape(1, HF).astype(np.float32), (P, 1))
        in_maps = []
        for k in range(n_cores):
            sd = plan.slot_dst[k]  # [NB, SC, P] node ids, -1 pads
            ere = np.full((NB, SC, P, 8), -1e4, np.float32)
            v = sd >= 0
            ere[v] = er_full[sd[v]]
            # dram layout [P, NB*SC*8]
            ere_arr = np.ascontiguousarray(
                ere.transpose(2, 0, 1, 3).reshape(P, NB * SC * 8)
            )
            im = {
                "idx": plan.idx[k],
                "dl": plan.dl[k],
                "ere": ere_arr,
                "iota": iota,
                "brep": b_rep,
            }
            for c in range(NCHUNK):
                im[f"z{c}"] = np.ascontiguousarray(
                    z_full[c * CH : c * CH + plan.chunk_rows[c]]
                )
            in_maps.append(im)
        res = _run(edge_nc, in_maps, n_cores)

        OW = n_classes if final else F
        nxt = np.zeros((N, OW), np.float32)
        for k in range(n_cores):
            r2n = plan.row2node[k]
            v = r2n >= 0
            nxt[r2n[v]] = res[k]["out"][v]
        if final:
            out_final = nxt
        else:
            h = nxt
    return out_final


def kernel(**inputs):
    x = np.asarray(inputs["x"], np.float32)
    src = np.asarray(inputs["src"])
    dst = np.asarray(inputs["dst"])
    params = []
    for i in range(3):
        params.append(
            {
                "W": np.asarray(inputs[f"W{i}"], np.float32),
                "al": np.asarray(inputs[f"al{i}"], np.float32),
                "ar": np.asarray(inputs[f"ar{i}"], np.float32),
                "b": np.asarray(inputs[f"b{i}"], np.float32),
            }
        )
    return gat_forward(x, src, dst, params, N=x.shape[0], n_cores=8,
                       n_classes=params[2]["al"].shape[1]).astype(np.float32)


# revision 18
# speedup vs baseline: 1.7129x; 1.7129x over previous
"""GAT (3-layer DGL-style GATConv) on 8 Trainium2 NeuronCores.

Strategy (graph/data parallel, dst-sharded):
  * dst nodes are sharded across the 8 cores (12500 each), grouped into
    128-dst blocks; per block, incoming edges are bucketed by src z-table
    chunk (int16 gather index range) into fixed-size slot groups.
  * Per layer, a "node" launch computes z = h @ W (plus the per-node
    attention terms el/er) sharded by node slice; the host concatenates the
    z slices (bf16 rows padded to a 256B-multiple, with el stored inside
    the row) and feeds the full z table to the "edge" launch.
  * The edge launch dma_gathers z rows by src edge-by-edge, computes
    ex = exp(leakyrelu(el_src + er_dst)) on-chip, and aggregates
    out[d] = sum_e ex_e * z[src_e] / sum_e ex_e with a mask matmul on the
    tensor engine (128-edge chunks against a 128-dst one-hot mask built
    with is_equal against an iota), accumulating numerator and denominator
    in PSUM in one pass.  Segment softmax needs no max subtraction (shift
    invariance; logits are O(8) here).
  * Head mean / ReLU / bias / final class softmax run fused in the edge
    launch epilogue.

kernel(**inputs) takes the FULL unsharded inputs and returns the FULL
[N, n_classes] float32 output.
"""

import math
import os
from dataclasses import dataclass, field

import numpy as np
import ml_dtypes

BF16 = ml_dtypes.bfloat16
P = 128
NCHUNK = 4  # z-table split so gather indices fit int16


# --------------------------------------------------------------------------
# host-side plan: dst->block packing, slot layout, gather index arrays
# --------------------------------------------------------------------------

@dataclass
class Plan:
    n_cores: int
    N: int
    ND: int          # dst nodes per core
    NB: int          # 128-dst blocks per core
    NT: int          # node tiles per core (= NB)
    CH: int          # z-chunk rows
    chunk_rows: list  # rows per z chunk
    G: int           # gather count per (block, chunk) group (mult of 128)
    SCc: int         # subchunks per chunk group (= G // 128)
    SC: int          # subchunks per block (= 4 * SCc)
    idx: list = field(default_factory=list)        # per core [P, NB*4*(G//16)] i16
    dl: list = field(default_factory=list)         # per core [P, NB*SC] bf16
    slot_dst: list = field(default_factory=list)   # per core [NB, SC, P] i32 (-1 pad)
    slot_src: list = field(default_factory=list)   # per core [NB, SC, P] i32 (-1 pad)
    row2node: list = field(default_factory=list)   # per core [NB*P] i32 (-1 pad)


def build_plan(src, dst, N, n_cores):
    src = np.asarray(src).astype(np.int64)
    dst = np.asarray(dst).astype(np.int64)
    ND = N // n_cores
    assert ND * n_cores == N
    NB = (ND + P - 1) // P
    CH = (N + NCHUNK - 1) // NCHUNK
    chunk_rows = [min(CH, N - c * CH) for c in range(NCHUNK)]

    cores = []
    gmax = 0
    for k in range(n_cores):
        m = (dst >= k * ND) & (dst < (k + 1) * ND)
        dk = dst[m] - k * ND
        sk = src[m]
        deg = np.bincount(dk, minlength=ND)
        order = np.argsort(-deg, kind="stable")
        blk = np.empty(ND, np.int32)
        pos = np.empty(ND, np.int32)
        # snake-deal dsts (desc degree) into NB blocks to balance edge counts
        for i in range(0, ND, NB):
            ch = order[i : i + NB]
            r = i // NB
            if r % 2 == 0:
                b_ids = np.arange(len(ch))
            else:
                b_ids = NB - 1 - np.arange(len(ch))
            blk[ch] = b_ids
            pos[ch] = r
        chunk_id = (sk // CH).astype(np.int64)
        cores.append((dk, sk, blk, pos, chunk_id))
        cnt = np.bincount(blk[dk] * NCHUNK + chunk_id, minlength=NB * NCHUNK)
        gmax = max(gmax, int(cnt.max()))

    G = ((gmax + P - 1) // P) * P
    SCc = G // P
    SC = NCHUNK * SCc
    plan = Plan(n_cores, N, ND, NB, NB, CH, chunk_rows, G, SCc, SC)

    GC = G // 16
    for k in range(n_cores):
        dk, sk, blk, pos, chunk_id = cores[k]
        idx_arr = np.zeros((P, NB * NCHUNK * GC), np.int16)
        dl_arr = np.zeros((P, NB * SC), BF16)
        slot_dst = np.full((NB, SC, P), -1, np.int32)
        slot_src = np.full((NB, SC, P), -1, np.int32)
        row2node = np.full(NB * P, -1, np.int32)

        # row2node: block b row p -> global dst node
        node_of = np.full((NB, P), -1, np.int64)
        node_of[blk, pos] = np.arange(ND)
        valid = node_of >= 0
        row2node[valid.ravel()] = (node_of[valid] + k * ND).astype(np.int32)

        # group edges by (block, chunk)
        key = blk[dk].astype(np.int64) * NCHUNK + chunk_id
        sort = np.argsort(key, kind="stable")
        ks = key[sort]
        dks = dk[sort]
        sks = sk[sort]
        starts = np.searchsorted(ks, np.arange(NB * NCHUNK))
        ends = np.searchsorted(ks, np.arange(NB * NCHUNK) + 1)
        for b in range(NB):
            for c in range(NCHUNK):
                g0, g1 = starts[b * NCHUNK + c], ends[b * NCHUNK + c]
                n = g1 - g0
                loc_idx = (sks[g0:g1] - c * CH).astype(np.int16)
                loc_dst = pos[dks[g0:g1]]
                # gather idxs: slot j -> [j%16, j//16] of a [16, GC] grid
                grid = np.zeros((16, GC), np.int16)
                flat = np.zeros(G, np.int16)
                flat[:n] = loc_idx
                grid[:, :] = flat.reshape(GC, 16).T
                idx_arr[:, (b * NCHUNK + c) * GC : (b * NCHUNK + c + 1) * GC] = (
                    np.tile(grid, (8, 1))
                )
                # slot s of group -> subchunk kk = c*SCc + s//128, partition s%128
                s = np.arange(n)
                kk = c * SCc + s // P
                pp = s % P
                dl_arr[pp, b * SC + kk] = loc_dst.astype(BF16)
                slot_dst[b, kk, pp] = dks[g0:g1] + k * ND
                slot_src[b, kk, pp] = sks[g0:g1]
        plan.idx.append(idx_arr)
        plan.dl.append(dl_arr)
        plan.slot_dst.append(slot_dst)
        plan.slot_src.append(slot_src)
        plan.row2node.append(row2node)
    return plan


# --------------------------------------------------------------------------
# bass program builders
# --------------------------------------------------------------------------

def _bass_mods():
    import concourse.bass as bass
    import concourse.bacc as bacc
    import concourse.tile as tile
    import concourse.mybir as mybir
    return bass, bacc, tile, mybir


def build_node_program(Din, HF, R, NT):
    """z = hT.T @ Wext.  Wext = [W | Wal | War] so el/er come out of the
    same matmul (el[n,h] = sum_f z[n,h,f] al[h,f] = h @ Wal, linear in h).
    z rows are bf16, width R, with el at cols [HF:HF+8], er at [HF+8:HF+16].
    Node tiles are processed in pairs to halve the DMA instruction count."""
    bass, bacc, tile, mybir = _bass_mods()
    f32, bf16 = mybir.dt.float32, mybir.dt.bfloat16
    KC = (Din + P - 1) // P
    assert NT % 2 == 0

    nc = bacc.Bacc("TRN2", target_bir_lowering=False, debug=False)
    hT = nc.dram_tensor("hT", [Din, NT * P], bf16, kind="ExternalInput").ap()
    W = nc.dram_tensor("W", [Din, HF + 16], bf16, kind="ExternalInput").ap()
    z_out = nc.dram_tensor("z_out", [NT * P, R], bf16, kind="ExternalOutput").ap()
    eo = nc.dram_tensor("eo", [NT * P, 16], bf16, kind="ExternalOutput").ap()

    with tile.TileContext(nc) as tc:
        from contextlib import ExitStack
        with ExitStack() as ctx:
            cpool = ctx.enter_context(tc.tile_pool(name="const", bufs=1))
            lpool = ctx.enter_context(tc.tile_pool(name="lhs", bufs=4))
            zpool = ctx.enter_context(tc.tile_pool(name="z", bufs=3))
            ppool = ctx.enter_context(tc.tile_pool(name="psum", bufs=2, space="PSUM"))

            W_t = []
            for kc in range(KC):
                K = min(P, Din - kc * P)
                wt = cpool.tile([K, HF + 16], bf16, tag=f"w{kc}")
                nc.sync.dma_start(wt[:], W[kc * P : kc * P + K, :])
                W_t.append(wt)

            zv = z_out.rearrange("(t p) r -> t p r", p=P)
            ev = eo.rearrange("(t p) r -> t p r", p=P)
            for tp in range(NT // 2):
                lhs = []
                for kc in range(KC):
                    K = min(P, Din - kc * P)
                    lh = lpool.tile([K, 2 * P], bf16, tag=f"lh{kc}")
                    nc.sync.dma_start(
                        lh[:], hT[kc * P : kc * P + K, tp * 2 * P : (tp + 1) * 2 * P]
                    )
                    lhs.append(lh)
                zrow = zpool.tile([P, 2, R], bf16, tag="zrow")
                et = zpool.tile([P, 2, 16], bf16, tag="et")
                for j in range(2):
                    ps = ppool.tile([P, HF], f32, tag=f"psz{j}")
                    pe = ppool.tile([P, 16], f32, tag=f"pse{j}")
                    for kc in range(KC):
                        nc.tensor.matmul(
                            ps[:], lhsT=lhs[kc][:, j * P : (j + 1) * P],
                            rhs=W_t[kc][:, 0:HF],
                            start=(kc == 0), stop=(kc == KC - 1),
                        )
                        nc.tensor.matmul(
                            pe[:], lhsT=lhs[kc][:, j * P : (j + 1) * P],
                            rhs=W_t[kc][:, HF : HF + 16],
                            start=(kc == 0), stop=(kc == KC - 1),
                        )
                    nc.scalar.activation(
                        zrow[:, j, 0:HF], ps[:], mybir.ActivationFunctionType.Copy
                    )
                    nc.vector.tensor_copy(out=et[:, j, :], in_=pe[:])
                    if R > HF:
                        nc.vector.memset(zrow[:, j, HF:R], 0)
                nc.sync.dma_start(zv[tp * 2 : tp * 2 + 2, :, :].transpose([1, 0, 2]),
                                  zrow[:])
                nc.sync.dma_start(ev[tp * 2 : tp * 2 + 2, :, :].transpose([1, 0, 2]),
                                  et[:])
    nc.compile()
    return nc


def build_edge_program(HF, R, NB, SC, SCc, G, chunk_rows, final, n_classes=41):
    """Gather z rows by src, segment-softmax aggregate per dst block.

    meta input per block (cols [b*MW : (b+1)*MW], bf16):
      [0 : SC*8]        el per slot (host-gathered el_full[src])
      [SC*8 : SC*16]    er per slot (host-gathered er_full[dst])
      [SC*16 : SC*17]   dst-local row per slot
    """
    bass, bacc, tile, mybir = _bass_mods()
    f32, bf16, i16 = mybir.dt.float32, mybir.dt.bfloat16, mybir.dt.int16
    H = 8
    F = HF // H
    GC = G // 16
    MW = SC * 17

    nqueues = int(os.environ.get("GAT_QUEUES", "4"))
    nc = bacc.Bacc("TRN2", target_bir_lowering=False, debug=False,
                   num_swdge_queues=nqueues)
    zc = [
        nc.dram_tensor(f"z{c}", [chunk_rows[c], R], bf16, kind="ExternalInput").ap()
        for c in range(NCHUNK)
    ]
    idx = nc.dram_tensor("idx", [P, NB * NCHUNK * GC], i16, kind="ExternalInput").ap()
    meta = nc.dram_tensor("meta", [P, NB * MW], bf16, kind="ExternalInput").ap()
    iota = nc.dram_tensor("iota", [P, P], bf16, kind="ExternalInput").ap()
    brep = nc.dram_tensor("brep", [P, HF], f32, kind="ExternalInput").ap()
    OW = n_classes if final else F
    out = nc.dram_tensor("out", [NB * P, OW], f32, kind="ExternalOutput").ap()

    with tile.TileContext(nc) as tc:
        from contextlib import ExitStack
        with ExitStack() as ctx:
            cpool = ctx.enter_context(tc.tile_pool(name="const", bufs=1))
            gpool = ctx.enter_context(tc.tile_pool(name="gath", bufs=2))
            mpool = ctx.enter_context(tc.tile_pool(name="mask", bufs=2))
            spool = ctx.enter_context(tc.tile_pool(name="small", bufs=3))
            opool = ctx.enter_context(tc.tile_pool(name="outs", bufs=3))
            ppool = ctx.enter_context(tc.tile_pool(name="psum", bufs=2, space="PSUM"))

            iota_t = cpool.tile([P, P], bf16, tag="iota")
            nc.sync.dma_start(iota_t[:], iota[:])
            b_t = cpool.tile([P, HF], f32, tag="brep")
            nc.sync.dma_start(b_t[:], brep[:])

            for b in range(NB):
                idx_t = spool.tile([P, NCHUNK * GC], i16, tag="idx")
                nc.sync.dma_start(
                    idx_t[:], idx[:, b * NCHUNK * GC : (b + 1) * NCHUNK * GC]
                )
                mt = spool.tile([P, MW], bf16, tag="meta")
                nc.sync.dma_start(mt[:], meta[:, b * MW : (b + 1) * MW])
                el_t = mt[:, 0 : SC * 8]
                er_t = mt[:, SC * 8 : SC * 16]
                dl_t = mt[:, SC * 16 : SC * 17]

                Zg = gpool.tile([P, SC, R], bf16, tag="Zg")
                for c in range(NCHUNK):
                    nc.gpsimd.dma_gather(
                        Zg[:, c * SCc : (c + 1) * SCc, :],
                        zc[c][:],
                        idx_t[:, c * GC : (c + 1) * GC],
                        num_idxs=G,
                        num_idxs_reg=G,
                        elem_size=R,
                        elem_step=R,
                        queue_num=c % nqueues,
                    )
                # e = el_src + er_dst
                e_t = spool.tile([P, SC * 8], bf16, tag="e")
                nc.vector.tensor_tensor(
                    out=e_t[:], in0=el_t, in1=er_t, op=mybir.AluOpType.add,
                )
                # leaky relu: max(e, 0.2*e)
                elr = spool.tile([P, SC * 8], bf16, tag="elr")
                nc.vector.scalar_tensor_tensor(
                    out=elr[:], in0=e_t[:], scalar=0.2, in1=e_t[:],
                    op0=mybir.AluOpType.mult, op1=mybir.AluOpType.max,
                )
                # ex, expanded to all F columns per head (ACT does exp+expand
                # in one pass so the DVE multiply below runs 2x on bf16)
                exF = mpool.tile([P, SC, HF], bf16, tag="exF")
                nc.scalar.activation(
                    exF[:].rearrange("p k (h f) -> p k h f", f=F),
                    elr[:]
                    .rearrange("p (k h) -> p k h", h=8)
                    .unsqueeze(3)
                    .to_broadcast([P, SC, 8, F]),
                    mybir.ActivationFunctionType.Exp,
                )
                exb = spool.tile([P, SC * 8], bf16, tag="exb")
                nc.scalar.activation(
                    exb[:], elr[:], mybir.ActivationFunctionType.Exp
                )
                # one-hot dst masks
                masks = mpool.tile([P, SC, P], bf16, tag="masks")
                nc.vector.tensor_tensor(
                    out=masks[:],
                    in0=dl_t[:].unsqueeze(2).to_broadcast([P, SC, P]),
                    in1=iota_t[:].unsqueeze(1).to_broadcast([P, SC, P]),
                    op=mybir.AluOpType.is_equal,
                )
                # scale gathered z rows by ex (in place, bf16 2x)
                nc.vector.tensor_tensor(
                    out=Zg[:, :, 0:HF],
                    in0=Zg[:, :, 0:HF],
                    in1=exF[:],
                    op=mybir.AluOpType.mult,
                )
                ps_n = ppool.tile([P, HF], f32, tag="psn")
                ps_s = ppool.tile([P, 8], f32, tag="pss")
                for k in range(SC):
                    nc.tensor.matmul(
                        ps_n[:], lhsT=masks[:, k, :], rhs=Zg[:, k, 0:HF],
                        start=(k == 0), stop=(k == SC - 1),
                    )
                    nc.tensor.matmul(
                        ps_s[:], lhsT=masks[:, k, :],
                        rhs=exb[:, k * 8 : (k + 1) * 8],
                        start=(k == 0), stop=(k == SC - 1),
                    )
                srm = spool.tile([P, 8], f32, tag="srm")
                nc.vector.tensor_scalar_max(out=srm[:], in0=ps_s[:], scalar1=1e-12)
                srec = spool.tile([P, 8], f32, tag="srec")
                nc.vector.reciprocal(out=srec[:], in_=srm[:])
                outg = opool.tile([P, HF], f32, tag="outg")
                nc.vector.tensor_tensor(
                    out=outg[:].rearrange("p (h f) -> p h f", f=F),
                    in0=ps_n[:].rearrange("p (h f) -> p h f", f=F),
                    in1=srec[:].unsqueeze(2).to_broadcast([P, 8, F]),
                    op=mybir.AluOpType.mult,
                )
                nc.vector.tensor_tensor(
                    out=outg[:], in0=outg[:], in1=b_t[:], op=mybir.AluOpType.add
                )
                if not final:
                    r = opool.tile([P, HF], f32, tag="r")
                    nc.scalar.activation(
                        r[:], outg[:], mybir.ActivationFunctionType.Relu,
                        scale=0.125,
                    )
                    ht = opool.tile([P, F], f32, tag="ht")
                    nc.vector.reduce_sum(
                        ht[:],
                        r[:].rearrange("p (h f) -> p f h", f=F),
                        axis=mybir.AxisListType.X,
                    )
                    nc.sync.dma_start(out[b * P : (b + 1) * P, :], ht[:])
                else:
                    q = opool.tile([P, n_classes], f32, tag="q")
                    nc.vector.reduce_sum(
                        q[:],
                        outg[:].rearrange("p (h f) -> p f h", f=F),
                        axis=mybir.AxisListType.X,
                    )
                    qm = spool.tile([P, 1], f32, tag="qm")
                    nc.vector.reduce_max(qm[:], q[:], axis=mybir.AxisListType.X)
                    negm = spool.tile([P, 1], f32, tag="negm")
                    nc.vector.tensor_scalar_mul(out=negm[:], in0=qm[:], scalar1=-0.125)
                    qe = opool.tile([P, n_classes], f32, tag="qe")
                    nc.scalar.activation(
                        qe[:], q[:], mybir.ActivationFunctionType.Exp,
                        bias=negm[:], scale=0.125,
                    )
                    qs = spool.tile([P, 1], f32, tag="qs")
                    nc.vector.reduce_sum(qs[:], qe[:], axis=mybir.AxisListType.X)
                    qsr = spool.tile([P, 1], f32, tag="qsr")
                    nc.vector.reciprocal(out=qsr[:], in_=qs[:])
                    outf = opool.tile([P, n_classes], f32, tag="outf")
                    nc.vector.tensor_single_scalar(
                        out=outf[:], in_=qe[:], scalar=qsr[:],
                        op=mybir.AluOpType.mult,
                    )
                    nc.sync.dma_start(out[b * P : (b + 1) * P, :], outf[:])
    nc.compile()
    return nc


# --------------------------------------------------------------------------
# orchestration
# --------------------------------------------------------------------------

_PROG_CACHE = {}
LAST_RUN_NS = []  # per-launch max-core exec ns when GAT_TRACE=1
LAST_RESULTS = []  # full BassKernelResults per launch when GAT_TRACE=1


def _get_prog(key, builder):
    if key not in _PROG_CACHE:
        _PROG_CACHE[key] = builder()
    return _PROG_CACHE[key]


def _run(nc, in_maps, n_cores):
    if os.environ.get("GAT_SIM", "0") == "1":
        return _run_sim(nc, in_maps)
    from concourse.bass_utils import run_bass_kernel_spmd

    trace = os.environ.get("GAT_TRACE", "0") == "1"
    core_ids = list(range(n_cores))
    res = run_bass_kernel_spmd(
        nc, in_maps, core_ids,
        trace=trace, trace_cores=core_ids if trace else None,
    )
    if trace:
        LAST_RUN_NS.append(res.exec_time_ns)
        LAST_RESULTS.append(res)
    return res.results


def _run_sim(nc, in_maps):
    """CoreSim (functional simulator) execution, one core at a time."""
    from concourse.bass_interp import CoreSim

    results = []
    for im in in_maps:
        sim = CoreSim(nc, trace=False, require_finite=False, require_nnan=False)
        for name, arr in im.items():
            sim.tensor(name)[:] = arr
        sim.simulate(check_with_hw=False)
        out = {}
        for alloc in nc.m.functions[0].allocations:
            import concourse.mybir as mybir
            if (
                isinstance(alloc, mybir.MemoryLocationSet)
                and alloc.kind == "ExternalOutput"
            ):
                name = alloc.memorylocations[0].name
                out[name] = np.array(sim.tensor(name))
        results.append(out)
    return results


def _row_pack(el_hf_bytes):
    pass


def gat_forward(x, src, dst, params, N=None, n_cores=8, n_classes=41):
    """params: list of 3 dicts with W [Din, H*F], al/ar [H, F], b [H, F]."""
    N = N if N is not None else x.shape[0]
    H = 8
    plan = build_plan(src, dst, N, n_cores)
    NB, NT, SC, SCc, G, CH = plan.NB, plan.NT, plan.SC, plan.SCc, plan.G, plan.CH
    GC = G // 16
    iota = np.tile(np.arange(P, dtype=np.float32).astype(BF16)[None, :], (P, 1))

    layer_dims = []
    for li, prm in enumerate(params):
        Din = prm["W"].shape[0]
        F = prm["al"].shape[1]
        HF = H * F
        # bf16 row: z padded to a 256-byte multiple
        R = ((HF * 2 + 255) // 256) * 256 // 2
        layer_dims.append((Din, F, HF, R))

    h = np.asarray(x, np.float32)
    out_final = None
    for li, prm in enumerate(params):
        Din, F, HF, R = layer_dims[li]
        final = li == len(params) - 1

        node_nc = _get_prog(
            ("node", Din, HF, R, NT), lambda: build_node_program(Din, HF, R, NT)
        )
        # fused weight: [W | Wal | War] so el/er come from the matmul
        W = prm["W"].astype(np.float32)
        Wal = np.einsum("khf,hf->kh", W.reshape(Din, H, F), prm["al"])
        War = np.einsum("khf,hf->kh", W.reshape(Din, H, F), prm["ar"])
        Wext = np.concatenate([W, Wal, War], axis=1).astype(BF16)
        in_maps = []
        for k in range(n_cores):
            hk = h[k * plan.ND : (k + 1) * plan.ND]
            hT = np.zeros((Din, NT * P), BF16)
            hT[:, : plan.ND] = hk.T.astype(BF16)
            in_maps.append({"hT": hT, "W": Wext})
        res = _run(node_nc, in_maps, n_cores)

        z_full = np.concatenate(
            [res[k]["z_out"][: plan.ND] for k in range(n_cores)], axis=0
        )
        eo_full = np.concatenate(
            [res[k]["eo"][: plan.ND] for k in range(n_cores)], axis=0
        ).astype(np.float32)
        el_full = eo_full[:, 0:8]
        er_full = eo_full[:, 8:16]

        edge_nc = _get_prog(
            ("edge", HF, R, NB, SC, G, final, tuple(plan.chunk_rows)),
            lambda: build_edge_program(
                HF, R, NB, SC, SCc, G, plan.chunk_rows, final, n_classes
            ),
        )
        b_rep = np.tile(prm["b"].reshape(1, HF).astype(np.float32), (P, 1))
        in_maps = []
        for k in range(n_cores):
            sd = plan.slot_dst[k]  # [NB, SC, P] node ids, -1 pads
            ss = plan.slot_src[k]
            v = sd >= 0
            ele = np.full((NB, SC, P, 8), -1e4, np.float32)
            ele[v] = el_full[ss[v]]
            ere = np.zeros((NB, SC, P, 8), np.float32)
            ere[v] = er_full[sd[v]]
            # meta layout per block: el [SC,8] | er [SC,8] | dl [SC]
            MW = SC * 17
            meta = np.empty((P, NB, MW), BF16)
            meta[:, :, 0 : SC * 8] = ele.transpose(2, 0, 1, 3).reshape(P, NB, SC * 8)
            meta[:, :, SC * 8 : SC * 16] = (
                ere.transpose(2, 0, 1, 3).reshape(P, NB, SC * 8)
            )
            meta[:, :, SC * 16 : SC * 17] = plan.dl[k].reshape(P, NB, SC)
            im = {
                "idx": plan.idx[k],
                "meta": np.ascontiguousarray(meta.reshape(P, NB * MW)),
                "iota": iota,
                "brep": b_rep,
            }
            for c in range(NCHUNK):
                im[f"z{c}"] = np.ascontiguousarray(
                    z_full[c * CH : c * CH + plan.chunk_rows[c]]
                )
            in_maps.append(im)
        res = _run(edge_nc, in_maps, n_cores)

        OW = n_classes if final else F
        nxt = np.zeros((N, OW), np.float32)
        for k in range(n_cores):
            r2n = plan.row2node[k]
            v = r2n >= 0
            nxt[r2n[v]] = res[k]["out"][v]
        if final:
            out_final = nxt
        else:
            h = nxt
    return out_final


def kernel(**inputs):
    x = np.asarray(inputs["x"], np.float32)
    src = np.asarray(inputs["src"])
    dst = np.asarray(inputs["dst"])
    params = []
    for i in range(3):
        params.append(
            {
                "W": np.asarray(inputs[f"W{i}"], np.float32),
                "al": np.asarray(inputs[f"al{i}"], np.float32),
                "ar": np.asarray(inputs[f"ar{i}"], np.float32),
                "b": np.asarray(inputs[f"b{i}"], np.float32),
            }
        )
    return gat_forward(x, src, dst, params, N=x.shape[0], n_cores=8,
                       n_classes=params[2]["al"].shape[1]).astype(np.float32)
